# revision 1
# baseline (speedup 1.0000x reference)
"""AttentionPool Trainium2 kernel: 8-core data-parallel Bass/Tile implementation.

Reference computation (per batch b of 32, S=2048, D=1024):
    xn = LayerNorm(x[b])                      # over D, eps 1e-5
    h = tanh(xn @ W1 + b1)
    scores = h @ W2 + b2                      # [S]
    w = softmax(scores)
    out[b] = sum_s w[s] * x[b, s, :]

Strategy: batch axis sharded over 8 cores (4 batches each). Per core:
  - LN stats on DVE (bn_stats/bn_aggr + Newton rsqrt), normalize to bf16
  - stage xn(bf16) to DRAM, read back transposed via DMA-xbar ([2048,128]->[128,2048])
  - matmul1 in bf16 (d-tiles accumulated in PSUM), tanh+c2 on ACT
  - scores matmul (M=1) in bf16, exp on ACT (scores are O(1): no max-subtract needed)
  - unnormalized pooling via f32r matmul against raw x, then divide by Z = sum(exp)
Host-side prep folds ln_gamma into W1 (W1p), ln_beta@W1+b1 into c2.
Phase 4 of batch b-1 is emitted between phases 1/2 and 3 of batch b (software
pipelining) so its cross-phase waits never head-of-line-block the next batch's
work on the DVE/GpSimd/Sync queues. Engine assignment (critical for avoiding
queue head-of-line blocking): GpSimd = x loads + f32r pooling re-reads;
Sync = xn stores + DMA transposes + output; Scalar(ACT) = tanh/exp/copies +
e-scatter bounces; DVE = LN stats (bn_stats/bn_aggr), Newton rsqrt, normalize.
"""
import sys
import os

sys.path.insert(0, '/opt/trn_rl_repo')

import numpy as np

import concourse.bass as bass
import concourse.tile as tile
from concourse import bacc, mybir
from concourse.bass_utils import run_bass_kernel_spmd

P = 128
D = 1024
S = 2048
B = 32
NCORES = 8
BLOC = B // NCORES            # batches per core
ROWS = BLOC * S               # 8192 rows per core
DT = D // P                   # 8 d-tiles
ET = D // P                   # 8 e-tiles
SUBT = S // P                 # 16 subtiles per batch
NG = 4                        # subtiles per normalize/Newton group
CHUNK = 512                   # matmul moving free dim
NCHUNK = S // CHUNK           # 4 chunks per batch

f32 = mybir.dt.float32
f32r = mybir.dt.float32r
bf16 = mybir.dt.bfloat16
AF = mybir.ActivationFunctionType
ALU = mybir.AluOpType


def build_nc():
    nc = bacc.Bacc("TRN2", target_bir_lowering=False, num_devices=NCORES)

    x = nc.dram_tensor("x", [ROWS, D], f32, kind="ExternalInput")
    w1p = nc.dram_tensor("w1p", [D, D], bf16, kind="ExternalInput")
    c2v = nc.dram_tensor("c2v", [D], f32, kind="ExternalInput")
    w2v = nc.dram_tensor("w2v", [D], bf16, kind="ExternalInput")
    b2s = nc.dram_tensor("b2s", [1, 1], f32, kind="ExternalInput")
    onev = nc.dram_tensor("onev", [P, 1], f32, kind="ExternalInput")
    out = nc.dram_tensor("out", [BLOC, D], f32, kind="ExternalOutput")

    with tile.TileContext(nc) as tc:
        with (
            tc.tile_pool(name="consts", bufs=1) as consts,
            tc.tile_pool(name="xnat", bufs=3) as xnat,        # [128,4,1024] f32
            tc.tile_pool(name="stats", bufs=6) as statp,
            tc.tile_pool(name="xnst", bufs=3) as xnst,        # [128,4,1024] bf16 staging
            tc.tile_pool(name="xnt", bufs=16) as xnt,         # [128,2048] bf16 transposed
            tc.tile_pool(name="hb", bufs=6) as hpool,         # [128,512] bf16
                        tc.tile_pool(name="sc", bufs=10) as scpool,       # small score/e tiles
            tc.tile_pool(name="xrd", bufs=2) as xrd,          # pooling re-read f32r
            tc.tile_pool(name="ob", bufs=2) as obuf,
            tc.tile_pool(name="psmm", bufs=4, space="PSUM") as psmm,
            tc.tile_pool(name="pssc", bufs=1, space="PSUM") as pssc,
            tc.tile_pool(name="pspool", bufs=2, space="PSUM") as pspool,
            tc.tile_pool(name="pz", bufs=1, space="PSUM") as pzpool,
            tc.tile_pool(name="dram", bufs=4, space="DRAM") as dramp,
        ):
            # ---- constants ----
            w1_sb = consts.tile([P, DT, D], bf16)        # [d_in_tile, d_tile, e]
            nc.scalar.dma_start(w1_sb, w1p.ap().rearrange("(t p) e -> p t e", p=P))
            c2_sb = consts.tile([P, ET], f32)            # [e_in_tile, e_tile]
            nc.scalar.dma_start(c2_sb, c2v.ap().rearrange("(t p) -> p t", p=P))
            w2_sb = consts.tile([P, ET], bf16)
            nc.scalar.dma_start(w2_sb, w2v.ap().rearrange("(t p) -> p t", p=P))
            b2_sb = consts.tile([1, 1], f32)
            nc.sync.dma_start(b2_sb, b2s.ap())
            x3 = x.ap().rearrange("(b t p) d -> b t p d", b=BLOC, p=P)
            ones_r = consts.tile([P, 1], f32r)

            def phase1(b, scratch, xts, split):
                """Load x, LN stats, normalize -> bf16, store to scratch;
                transposes are emitted as soon as their source rows are stored."""
                scr3 = scratch.rearrange("(t p) d -> t p d", p=P)
                HS = S // 2
                for grp in range(SUBT // NG):
                    xt = xnat.tile([P, NG, D], f32, tag="xnat")
                    t0 = grp * NG
                    if b == 0 and grp == 0:
                        for s in range(NG):
                            nc.gpsimd.dma_start(
                                xt[:, s:s + 1, :],
                                x3[b, t0 + s:t0 + s + 1].rearrange(
                                    "t p d -> p t d"))
                    else:
                        nc.gpsimd.dma_start(
                            xt, x3[b, t0:t0 + NG].rearrange("t p d -> p t d"))
                    mv = statp.tile([P, NG, 2], f32, tag="mv")
                    for s in range(NG):
                        st = statp.tile([P, 2, 6], f32, tag="bnst")
                        nc.vector.bn_stats(st[:, 0, :], xt[:, s, 0:512])
                        nc.vector.bn_stats(st[:, 1, :], xt[:, s, 512:1024])
                        nc.vector.bn_aggr(mv[:, s, :], st)
                    # rstd = rsqrt(var+eps): quake seed + 2 Newton steps (DVE only)
                    var = statp.tile([P, NG], f32, tag="var")
                    nc.vector.tensor_scalar(out=var, in0=mv[:, :, 1],
                                            scalar1=1e-5, scalar2=0.5,
                                            op0=ALU.add, op1=ALU.mult)
                    y = statp.tile([P, NG], f32, tag="y")
                    yi = y.bitcast(mybir.dt.int32)
                    vi = var.bitcast(mybir.dt.int32)
                    nc.vector.tensor_scalar(out=yi, in0=vi, scalar1=0x800000,
                                            scalar2=None, op0=ALU.add)
                    nc.vector.tensor_scalar(out=yi, in0=yi, scalar1=1,
                                            scalar2=None,
                                            op0=ALU.logical_shift_right)
                    nc.vector.tensor_scalar(out=yi, in0=yi, scalar1=-1,
                                            scalar2=0x5f3759df,
                                            op0=ALU.mult, op1=ALU.add)
                    tny = statp.tile([P, NG], f32, tag="tny")
                    for _ in range(2):
                        nc.vector.tensor_tensor(tny, y, y, ALU.mult)
                        nc.vector.tensor_tensor(tny, tny, var, ALU.mult)
                        nc.vector.tensor_scalar(out=tny, in0=tny, scalar1=-1.0,
                                                scalar2=1.5,
                                                op0=ALU.mult, op1=ALU.add)
                        nc.vector.tensor_tensor(y, y, tny, ALU.mult)
                    xnb = xnst.tile([P, NG, D], bf16, tag="xnst")
                    for s in range(NG):
                        nc.vector.tensor_scalar(out=xnb[:, s, :], in0=xt[:, s, :],
                                                scalar1=mv[:, s, 0:1],
                                                scalar2=y[:, s:s + 1],
                                                op0=ALU.subtract, op1=ALU.mult)
                    nc.sync.dma_start(
                        scr3[t0:t0 + NG].rearrange("t p d -> p t d"), xnb)
                    # transposes whose source rows are now fully stored
                    if split:
                        # quarter q == group grp (NG*128 == CHUNK rows)
                        for d in range(DT):
                            hh, q = grp // 2, grp % 2
                            nc.sync.dma_start_transpose(
                                xts[hh][d][:, q * CHUNK:(q + 1) * CHUNK],
                                scratch[grp * CHUNK:(grp + 1) * CHUNK,
                                        d * P:(d + 1) * P])
                    elif grp % 2 == 1:
                        hh = grp // 2
                        for d in range(DT):
                            nc.sync.dma_start_transpose(
                                xts[hh][d],
                                scratch[hh * HS:(hh + 1) * HS,
                                        d * P:(d + 1) * P])

            def alloc_xts():
                xts = []
                for hh in range(2):
                    row = []
                    for _d in range(DT):
                        xT = xnt.tile([P, S // 2], bf16, tag="xnt", name="xnt_t")
                        row.append(xT)
                    xts.append(row)
                return xts

            def scatter_chunk(ec):
                """Bounce one chunk's exp scores to packed [128, 4] f32r."""
                ebounce = dramp.tile([1, CHUNK], f32, tag="eb", name="eb")
                nc.scalar.dma_start(ebounce, ec)
                epk_f = scpool.tile([P, NCHUNK], f32, tag="epk_f", name="epk_f")
                nc.scalar.dma_start(
                    epk_f, ebounce.rearrange("o (t p) -> (o p) t", p=P))
                epk = scpool.tile([P, NCHUNK], f32r, tag="epk", name="epk")
                nc.vector.tensor_copy(epk, epk_f)
                return epk

            def phase3(b, xts, last):
                """matmul1 + tanh + scores + exp per chunk.

                For the last batch the scatter AND pooling matmuls are
                emitted inline per chunk (nothing follows that the DVE
                copy could block), shrinking the kernel tail; earlier
                batches return plain [1, CHUNK] exp tiles scattered in
                phase4 to keep cross-queue ordering clean.
                """
                if last:
                    zp = pzpool.tile([1, CHUNK], f32, tag="pz", name="zp")
                    pp0 = pspool.tile([1, CHUNK], f32, tag="pspool", name="pp0")
                    pp1 = pspool.tile([1, CHUNK], f32, tag="pspool", name="pp1")
                eouts = []
                for c in range(NCHUNK):
                    ps_sc = pssc.tile([1, CHUNK], f32, tag="pssc")
                    for e in range(ET):
                        ps = psmm.tile([P, CHUNK], f32, tag="psmm")
                        for d in range(DT):
                            nc.tensor.matmul(
                                ps, w1_sb[:, d, e * P:(e + 1) * P],
                                xts[c // 2][d][:, (c % 2) * CHUNK:
                                               (c % 2 + 1) * CHUNK],
                                start=(d == 0), stop=(d == DT - 1))
                        ht = hpool.tile([P, CHUNK], bf16, tag="hb")
                        nc.scalar.activation(ht, ps, AF.Tanh,
                                             bias=c2_sb[:, e:e + 1])
                        nc.tensor.matmul(ps_sc, w2_sb[:, e:e + 1], ht,
                                         start=(e == 0), stop=(e == ET - 1))
                    ec = scpool.tile([1, CHUNK], f32, tag="ec", name="ec")
                    nc.scalar.activation(ec, ps_sc, AF.Exp, bias=b2_sb[0:1, 0:1])
                    if last:
                        epk = scatter_chunk(ec)
                        xq = xrd.tile([P, 4, D], f32r, tag="xrd", name="xq")
                        nc.gpsimd.dma_start(
                            xq,
                            x3[b, c * 4:(c + 1) * 4].rearrange("t p d -> p t d"))
                        nc.tensor.matmul(zp[:, 0:NCHUNK], ones_r, epk,
                                         start=(c == 0), stop=(c == 3))
                        for s in range(4):
                            t = c * 4 + s
                            nc.tensor.matmul(pp0, epk[:, s:s + 1],
                                             xq[:, s, 0:512],
                                             start=(t == 0), stop=(t == SUBT - 1))
                            nc.tensor.matmul(pp1, epk[:, s:s + 1],
                                             xq[:, s, 512:1024],
                                             start=(t == 0), stop=(t == SUBT - 1))
                        eouts = (zp, pp0, pp1)
                    else:
                        eouts.append(ec)
                return eouts

            def phase4(b, eouts):
                """Pooling matmuls per chunk, Z via tiny matmuls, output."""
                if isinstance(eouts, tuple):
                    zp, pp0, pp1 = eouts       # last batch: pooling done inline
                else:
                    epks = [scatter_chunk(ec) for ec in eouts]
                    zp = pzpool.tile([1, CHUNK], f32, tag="pz", name="zp")
                    pp0 = pspool.tile([1, CHUNK], f32, tag="pspool")
                    pp1 = pspool.tile([1, CHUNK], f32, tag="pspool")
                    for q in range(4):
                        xq = xrd.tile([P, 4, D], f32r, tag="xrd")
                        nc.gpsimd.dma_start(
                            xq,
                            x3[b, q * 4:(q + 1) * 4].rearrange("t p d -> p t d"))
                        nc.tensor.matmul(zp[:, 0:NCHUNK], ones_r, epks[q],
                                         start=(q == 0), stop=(q == 3))
                        for s in range(4):
                            t = q * 4 + s
                            nc.tensor.matmul(pp0, epks[q][:, s:s + 1],
                                             xq[:, s, 0:512],
                                             start=(t == 0),
                                             stop=(t == SUBT - 1))
                            nc.tensor.matmul(pp1, epks[q][:, s:s + 1],
                                             xq[:, s, 512:1024],
                                             start=(t == 0),
                                             stop=(t == SUBT - 1))
                zt = scpool.tile([1, 1], f32, tag="zt")
                nc.vector.tensor_reduce(zt, zp[:, 0:NCHUNK],
                                        axis=mybir.AxisListType.X, op=ALU.add)
                rz = scpool.tile([1, 1], f32, tag="rz")
                nc.vector.reciprocal(rz, zt)
                ob = obuf.tile([1, D], f32, tag="ob")
                nc.scalar.activation(ob[:, 0:512], pp0, AF.Copy,
                                     scale=rz[0:1, 0:1])
                nc.scalar.activation(ob[:, 512:1024], pp1, AF.Copy,
                                     scale=rz[0:1, 0:1])
                nc.sync.dma_start(out.ap()[b:b + 1, :], ob)

            prev = None   # (b, escore) of previous batch
            for b in range(BLOC):
                scratch = dramp.tile([S, D], bf16, tag="scratch")
                _mark(nc, f"ph1_b{b}")
                xts = alloc_xts()
                phase1(b, scratch, xts, split=(b == 0))
                if b == 0:
                    # needed first by phase4(b0); keep it off the queue head
                    nc.gpsimd.dma_start(ones_r, onev.ap())
                if prev is not None:
                    _mark(nc, f"ph4_b{prev[0]}")
                    phase4(*prev)
                _mark(nc, f"ph3_b{b}")
                epks = phase3(b, xts, last=(b == BLOC - 1))
                prev = (b, epks)
            _mark(nc, f"ph4_b{BLOC - 1}")
            phase4(*prev)
            _mark(nc, "end")

    nc.compile()
    return nc


PHASE_MARKS = []   # (inst_counter_at_phase_start, phase_name)


def _mark(nc, name):
    n = nc.get_next_instruction_name()   # consumes one name: I-<k>
    PHASE_MARKS.append((int(n.split('-')[1]), name))


_NC_CACHE = {}


def _get_nc():
    if "nc" not in _NC_CACHE:
        _NC_CACHE["nc"] = build_nc()
    return _NC_CACHE["nc"]


def _prep_host(ln_gamma, ln_beta, W1, b1, W2, b2):
    import ml_dtypes
    W1p = (np.asarray(ln_gamma, np.float32)[:, None]
           * np.asarray(W1, np.float32)).astype(ml_dtypes.bfloat16)
    c2 = (np.asarray(ln_beta, np.float32) @ np.asarray(W1, np.float32)
          + np.asarray(b1, np.float32))
    w2v = np.ascontiguousarray(
        np.asarray(W2, np.float32)[:, 0]).astype(ml_dtypes.bfloat16)
    b2s = np.asarray(b2, np.float32).reshape(1, 1)
    return np.ascontiguousarray(W1p), np.ascontiguousarray(c2), w2v, b2s


def run_cores(inputs, trace=False, **kw):
    x = np.asarray(inputs["x"], np.float32)
    W1p, c2, w2v, b2s = _prep_host(inputs["ln_gamma"], inputs["ln_beta"],
                                   inputs["W1"], inputs["b1"],
                                   inputs["W2"], inputs["b2"])
    nc = _get_nc()
    in_maps = []
    for c in range(NCORES):
        shard = np.ascontiguousarray(
            x[c * BLOC:(c + 1) * BLOC].reshape(ROWS, D))
        in_maps.append(dict(x=shard, w1p=W1p, c2v=c2, w2v=w2v, b2s=b2s,
                            onev=np.ones((P, 1), np.float32)))
    res = run_bass_kernel_spmd(nc, in_maps, core_ids=list(range(NCORES)),
                               trace=trace, **kw)
    full = np.concatenate([res.results[c]["out"] for c in range(NCORES)], axis=0)
    return full, res


def kernel(**inputs) -> np.ndarray:
    out, _ = run_cores(inputs, trace=False)
    return out.astype(np.float32)



# revision 3
# speedup vs baseline: 1.3880x; 1.3880x over previous
"""AttentionPool Trainium2 kernel: 8-core data-parallel Bass/Tile implementation.

Reference computation (per batch b of 32, S=2048, D=1024):
    xn = LayerNorm(x[b])                      # over D, eps 1e-5
    h = tanh(xn @ W1 + b1)
    scores = h @ W2 + b2                      # [S]
    w = softmax(scores)
    out[b] = sum_s w[s] * x[b, s, :]

Strategy: batch axis sharded over 8 cores (4 batches each). Per core/batch:
  - x loaded once with an f32->bf16 cast during the SWDGE DMA; the bf16
    natural-layout copy stays in SBUF and later feeds the pooling matmuls
    (no second HBM read of x).
  - LN stats via bn_stats/bn_aggr on the bf16 copy + Newton rsqrt (DVE),
    normalize with one fused tensor_scalar per subtile (bf16 fast path).
  - xn written to a DRAM scratch with a bf16->fp8e4 cast (SWDGE); read
    back via DMA-xbar transpose at 2-byte granularity, which yields fp8
    PAIRS per partition -- exactly the [K,2,N] moving-operand layout that
    MatmulPerfMode.DoubleRow contracts over (256-deep K per matmul).
  - matmul1 in fp8 DoubleRow (2x PE throughput), weights pre-packed on
    host in (super-tile, partition, plane) order and scaled by 32 so W1
    uses the e4m3 range; the 1/32 undo rides the tanh activation's scale.
  - tanh+c2 on ACT over paired [128,1024] PSUM tiles; scores matmul in
    bf16, all 4 chunks accumulated in ONE PSUM bank at partitions
    0/32/64/96 (tile_position col-tiling), emitted one e-group late so
    ACT latency never stalls the in-order PE queue.
  - b2 is dropped: softmax is invariant to a uniform score shift.
  - exp scores bounce through DRAM to repack [2048] -> [128,16]; Z comes
    from a [1,2048] re-read + DVE reduce (keeps everything partition-0).
  - pooling matmuls in bf16 against the SBUF-resident x copy, d-halves
    done sequentially through one PSUM row (bank budget: 6+1+1 = 8).
Batches are software-pipelined: phase1(b+1) (load/LN/quarter-transposes)
overlaps phase3(b) (matmuls); pooling of b-1 is sandwiched after the
first two e-groups of batch b so the scatter round-trip stays hidden.
Host-side prep folds ln_gamma into W1 and ln_beta@W1+b1 into c2.
"""
import sys
import os

sys.path.insert(0, '/opt/trn_rl_repo')

import numpy as np

import concourse.bass as bass
import concourse.tile as tile
from concourse import bacc, mybir
from concourse.bass_utils import run_bass_kernel_spmd

P = 128
D = 1024
S = 2048
B = 32
NCORES = 8
BLOC = B // NCORES            # batches per core
ROWS = BLOC * S               # 8192 rows per core
SUBT = S // P                 # 16 subtiles per batch
NG = 4                        # subtiles per stats/normalize group (= quarter)
CHUNK = 512                   # matmul moving free dim (output cols)
NCHUNK = S // CHUNK           # 4 chunks per batch
ET = D // P                   # 8 e-tiles

FP8 = True                    # matmul1 via fp8 DoubleRow
W1SCALE = 32.0                # host scales W1 by this; undone in tanh's scale
KT = 4 if FP8 else 8          # contraction super-tiles for matmul1
NPT = 4 if FP8 else 8         # transposed partition-tiles per batch

f32 = mybir.dt.float32
bf16 = mybir.dt.bfloat16
fp8 = mybir.dt.float8e4
AF = mybir.ActivationFunctionType
ALU = mybir.AluOpType
DR = mybir.MatmulPerfMode.DoubleRow


def build_nc():
    nc = bacc.Bacc("TRN2", target_bir_lowering=False, num_devices=NCORES)

    x = nc.dram_tensor("x", [ROWS, D], f32, kind="ExternalInput")
    if FP8:
        w1p = nc.dram_tensor("w1p", [P, KT, 2, ET, P], fp8,
                             kind="ExternalInput")
    else:
        w1p = nc.dram_tensor("w1p", [P, KT, ET, P], bf16,
                             kind="ExternalInput")
    c2v = nc.dram_tensor("c2v", [D], f32, kind="ExternalInput")
    w2v = nc.dram_tensor("w2v", [D], bf16, kind="ExternalInput")
    out = nc.dram_tensor("out", [BLOC, D], f32, kind="ExternalOutput")

    with tile.TileContext(nc) as tc:
        with (
            tc.tile_pool(name="consts", bufs=1) as consts,
            tc.tile_pool(name="xa", bufs=2) as xap,        # [128,16,1024] bf16
            tc.tile_pool(name="stats", bufs=8) as statp,
            tc.tile_pool(name="xnst", bufs=3) as xnst,     # [128,4,1024] bf16
            tc.tile_pool(name="xt", bufs=2) as xtp,        # [128,NPT,2048] bf16
            tc.tile_pool(name="ht", bufs=4) as htp,        # [128,1024] bf16
            tc.tile_pool(name="sc", bufs=5) as scp,        # small score tiles
            tc.tile_pool(name="ob", bufs=2) as obp,
            tc.tile_pool(name="psmm", bufs=3, space="PSUM") as psmm,  # 2 banks
            tc.tile_pool(name="pssc", bufs=1, space="PSUM") as pssc,  # 1 bank
            tc.tile_pool(name="pspl", bufs=1, space="PSUM") as pspl,  # 1 bank
            tc.tile_pool(name="dram", bufs=4, space="DRAM") as dramp,
        ):
            # ---- constants ----
            w1_sb = consts.tile(list(w1p.shape), fp8 if FP8 else bf16)
            nc.scalar.dma_start(w1_sb, w1p.ap())
            c2_sb = consts.tile([P, ET], f32)
            nc.scalar.dma_start(c2_sb, c2v.ap().rearrange("(t p) -> p t", p=P))
            w2_sb = consts.tile([P, ET], bf16)
            nc.scalar.dma_start(w2_sb, w2v.ap().rearrange("(t p) -> p t", p=P))
            ones_sb = consts.tile([P, 1], bf16)
            nc.vector.memset(ones_sb, 1.0)
            x3 = x.ap().rearrange("(b t p) d -> b t p d", b=BLOC, p=P)

            def phase1(b, xa, xtt, scratch):
                """Load+cast x, LN stats, normalize -> fp8 scratch, quarter
                transposes as soon as each quarter's rows are staged."""
                scrT = scratch.bitcast(bf16)          # [S, D//2] pair view
                scr3 = scratch.rearrange("(t p) d -> t p d", p=P)
                for g in range(NG):
                    t0 = NG * g
                    if b == 0:
                        for s2 in range(0, NG, 2):
                            nc.gpsimd.dma_start(
                                xa[:, t0 + s2:t0 + s2 + 2, :],
                                x3[b, t0 + s2:t0 + s2 + 2].rearrange(
                                    "t p d -> p t d"))
                    else:
                        nc.gpsimd.dma_start(
                            xa[:, t0:t0 + NG, :],
                            x3[b, t0:t0 + NG].rearrange("t p d -> p t d"))
                    mv = statp.tile([P, NG, 2], f32, tag="mv")
                    for s in range(NG):
                        st = statp.tile([P, 2, 6], f32, tag="bnst")
                        nc.vector.bn_stats(st[:, 0, :], xa[:, t0 + s, 0:512])
                        nc.vector.bn_stats(st[:, 1, :], xa[:, t0 + s, 512:1024])
                        nc.vector.bn_aggr(mv[:, s, :], st)
                    # rstd = rsqrt(var+eps): quake seed + 2 Newton steps (DVE)
                    var = statp.tile([P, NG], f32, tag="var")
                    nc.vector.tensor_scalar(out=var, in0=mv[:, :, 1],
                                            scalar1=1e-5, scalar2=0.5,
                                            op0=ALU.add, op1=ALU.mult)
                    y = statp.tile([P, NG], f32, tag="y")
                    yi = y.bitcast(mybir.dt.int32)
                    vi = var.bitcast(mybir.dt.int32)
                    nc.vector.tensor_scalar(out=yi, in0=vi, scalar1=0x800000,
                                            scalar2=None, op0=ALU.add)
                    nc.vector.tensor_scalar(out=yi, in0=yi, scalar1=1,
                                            scalar2=None,
                                            op0=ALU.logical_shift_right)
                    nc.vector.tensor_scalar(out=yi, in0=yi, scalar1=-1,
                                            scalar2=0x5f3759df,
                                            op0=ALU.mult, op1=ALU.add)
                    tny = statp.tile([P, NG], f32, tag="tny")
                    for _ in range(2):
                        nc.vector.tensor_tensor(tny, y, y, ALU.mult)
                        nc.vector.tensor_tensor(tny, tny, var, ALU.mult)
                        nc.vector.tensor_scalar(out=tny, in0=tny, scalar1=-1.0,
                                                scalar2=1.5,
                                                op0=ALU.mult, op1=ALU.add)
                        nc.vector.tensor_tensor(y, y, tny, ALU.mult)
                    xnb = xnst.tile([P, NG, D], bf16, tag="xnst")
                    for s in range(NG):
                        nc.vector.tensor_scalar(out=xnb[:, s, :],
                                                in0=xa[:, t0 + s, :],
                                                scalar1=mv[:, s, 0:1],
                                                scalar2=y[:, s:s + 1],
                                                op0=ALU.subtract, op1=ALU.mult)
                    nc.gpsimd.dma_start(
                        scr3[t0:t0 + NG].rearrange("t p d -> p t d"), xnb)
                    # quarter g rows are now staged: transpose them
                    for t in range(NPT):
                        nc.sync.dma_start_transpose(
                            xtt[:, t, g * CHUNK:(g + 1) * CHUNK],
                            scrT[g * CHUNK:(g + 1) * CHUNK,
                                 t * P:(t + 1) * P])

            def emit_pool_half(pl_ps, epk, xa, half, c=None):
                """Pooling matmuls for d-half `half`, subtiles of chunk c
                (or all 16)."""
                d0 = half * 512
                rng = range(4 * c, 4 * c + 4) if c is not None else range(SUBT)
                for t in rng:
                    s = (t - 4 * c) if c is not None else t
                    nc.tensor.matmul(pl_ps, epk[:, s:s + 1],
                                     xa[:, t, d0:d0 + 512],
                                     start=(t == 0), stop=(t == SUBT - 1))

            def z_chain(eb):
                """Z = sum(exp scores) via [1,2048] re-read + DVE reduce."""
                ztv = scp.tile([1, S], f32, tag="ztv")
                nc.scalar.dma_start(ztv, eb.rearrange("(a s) -> a s", a=1))
                zt = scp.tile([1, 1], f32, tag="zt")
                nc.vector.tensor_reduce(zt, ztv, axis=mybir.AxisListType.X,
                                        op=ALU.add)
                rz = scp.tile([1, 1], f32, tag="rz")
                nc.vector.reciprocal(rz, zt)
                return rz

            def pool_store(b, pl_ps, rz, half):
                obt = obp.tile([1, 512], f32, tag=f"ob{half}")
                nc.scalar.activation(obt, pl_ps, AF.Copy, scale=rz[0:1, 0:1])
                nc.sync.dma_start(
                    out.ap()[b:b + 1, half * 512:half * 512 + 512], obt)

            def phase4(b, epk_f, eb, xa):
                """Batch-level pooling for a non-last batch."""
                rz = z_chain(eb)
                epk = scp.tile([P, SUBT], bf16, tag="epk")
                nc.vector.tensor_copy(epk, epk_f)
                pl_ps = pspl.tile([1, 512], f32, tag="pspl")
                for half in range(2):
                    emit_pool_half(pl_ps, epk, xa, half)
                    pool_store(b, pl_ps, rz, half)

            def phase3_pass(b, xa, xtt, pairs, sc_ps, prev):
                """matmul1 + tanh + scores for the chunk-pairs in `pairs`."""
                f8 = xtt.bitcast(fp8) if FP8 else None   # [128,KT,4096]
                hts = [None] * ET

                def rhs(t, c):
                    if FP8:
                        return f8[:, t, c * 2 * CHUNK:(c + 1) * 2 * CHUNK] \
                            .rearrange("p (s two) -> p two s", two=2)
                    return xtt[:, t, c * CHUNK:(c + 1) * CHUNK]

                def lhs(t, e):
                    if FP8:
                        return w1_sb[:, t, :, e, :]
                    return w1_sb[:, t, e, :]

                def emit_sc(e):
                    for hti, (ca, cb) in zip(hts[e], pairs):
                        for c in (ca, cb):
                            nc.tensor.matmul(
                                sc_ps[32 * c:32 * c + 1, :], w2_sb[:, e:e + 1],
                                hti[:, (c % 2) * CHUNK:(c % 2 + 1) * CHUNK],
                                start=(e == 0), stop=(e == ET - 1),
                                tile_position=(0, 32 * c))

                pm = DR if FP8 else None
                tanh_scale = (1.0 / W1SCALE) if FP8 else 1.0
                for e in range(ET):
                    etiles = []
                    for (ca, cb) in pairs:
                        ps = psmm.tile([P, 2 * CHUNK], f32, tag="mm")
                        for t in range(KT):
                            nc.tensor.matmul(ps[:, 0:CHUNK], lhs(t, e),
                                             rhs(t, ca),
                                             start=(t == 0), stop=(t == KT - 1),
                                             perf_mode=pm)
                            nc.tensor.matmul(ps[:, CHUNK:2 * CHUNK], lhs(t, e),
                                             rhs(t, cb),
                                             start=(t == 0), stop=(t == KT - 1),
                                             perf_mode=pm)
                        hti = htp.tile([P, 2 * CHUNK], bf16, tag="ht")
                        nc.scalar.activation(hti, ps, AF.Tanh,
                                             bias=c2_sb[:, e:e + 1],
                                             scale=tanh_scale)
                        etiles.append(hti)
                    hts[e] = etiles
                    if e >= 1:
                        emit_sc(e - 1)
                    if e == 1 and prev is not None:
                        phase4(*prev)
                        prev = None
                emit_sc(ET - 1)
                return prev

            def phase3(b, xa, xtt, prev):
                """Full phase3: matmul passes, exp + scatter (+ inline
                pooling for the last batch)."""
                last = (b == BLOC - 1)
                sc_ps = pssc.tile([P, CHUNK], f32, tag="pssc")
                if b == 0:
                    # quarters stream in: run chunk-pairs as separate passes
                    phase3_pass(b, xa, xtt, [(0, 1)], sc_ps, None)
                    phase3_pass(b, xa, xtt, [(2, 3)], sc_ps, None)
                else:
                    prev = phase3_pass(b, xa, xtt, [(0, 1), (2, 3)], sc_ps,
                                       prev)
                    assert prev is None

                ec = scp.tile([P, CHUNK], f32, tag="ec")
                eb = dramp.tile([S], f32, tag="eb")
                if not last:
                    for c in range(NCHUNK):
                        nc.scalar.activation(ec[32 * c:32 * c + 1, :],
                                             sc_ps[32 * c:32 * c + 1, :],
                                             AF.Exp)
                    nc.scalar.dma_start(
                        eb.rearrange("(c j) -> c j", c=NCHUNK),
                        ec.rearrange("(a b) f -> a b f", b=32)[:, 0, :])
                    epk_f = scp.tile([P, SUBT], f32, tag="epkf")
                    nc.scalar.dma_start(
                        epk_f, eb.rearrange("(t p) -> p t", p=P))
                    return (b, epk_f, eb, xa)

                # last batch: per-chunk scatter + inline pooling
                epks = []
                pl_ps = pspl.tile([1, 512], f32, tag="pspl")
                for c in range(NCHUNK):
                    nc.scalar.activation(ec[32 * c:32 * c + 1, :],
                                         sc_ps[32 * c:32 * c + 1, :],
                                         AF.Exp)
                    nc.scalar.dma_start(eb[c * CHUNK:(c + 1) * CHUNK],
                                        ec[32 * c:32 * c + 1, :])
                    epk_f = scp.tile([P, NCHUNK], f32, tag="epkf")
                    nc.scalar.dma_start(
                        epk_f,
                        eb[c * CHUNK:(c + 1) * CHUNK].rearrange(
                            "(t p) -> p t", p=P))
                    epk = scp.tile([P, NCHUNK], bf16, tag="epk")
                    nc.vector.tensor_copy(epk, epk_f)
                    epks.append(epk)
                    emit_pool_half(pl_ps, epk, xa, 0, c=c)
                rz = z_chain(eb)
                pool_store(b, pl_ps, rz, 0)
                for c in range(NCHUNK):
                    emit_pool_half(pl_ps, epks[c], xa, 1, c=c)
                pool_store(b, pl_ps, rz, 1)
                return None

            prev = None
            for b in range(BLOC):
                xa = xap.tile([P, SUBT, D], bf16, tag="xa", name=f"xa{b}")
                xtt = xtp.tile([P, NPT, S], bf16, tag="xt", name=f"xt{b}")
                scratch = dramp.tile([S, D], fp8 if FP8 else bf16,
                                     tag="scratch")
                phase1(b, xa, xtt, scratch)
                prev = phase3(b, xa, xtt, prev)
            assert prev is None

    nc.compile()
    return nc


_NC_CACHE = {}


def _get_nc():
    if "nc" not in _NC_CACHE:
        _NC_CACHE["nc"] = build_nc()
    return _NC_CACHE["nc"]


def _prep_host(ln_gamma, ln_beta, W1, b1, W2, b2):
    import ml_dtypes
    W1g = (np.asarray(ln_gamma, np.float32)[:, None]
           * np.asarray(W1, np.float32))
    if FP8:
        # pack rows in DoubleRow (super-tile, partition, plane) order:
        # d = t*256 + p*2 + i  ->  arr[p, t, i, e8, e128]
        W1s = (W1g * W1SCALE).astype(ml_dtypes.float8_e4m3)
        W1pk = np.ascontiguousarray(
            W1s.reshape(KT, P, 2, ET, P).transpose(1, 0, 2, 3, 4))
    else:
        # d = t*128 + p  ->  arr[p, t, e8, e128]
        W1s = W1g.astype(ml_dtypes.bfloat16)
        W1pk = np.ascontiguousarray(
            W1s.reshape(KT, P, ET, P).transpose(1, 0, 2, 3))
    c2 = (np.asarray(ln_beta, np.float32) @ np.asarray(W1, np.float32)
          + np.asarray(b1, np.float32))
    w2v = np.ascontiguousarray(
        np.asarray(W2, np.float32)[:, 0]).astype(ml_dtypes.bfloat16)
    return W1pk, np.ascontiguousarray(c2), w2v


def run_cores(inputs, trace=False, **kw):
    x = np.asarray(inputs["x"], np.float32)
    W1pk, c2, w2v = _prep_host(inputs["ln_gamma"], inputs["ln_beta"],
                               inputs["W1"], inputs["b1"],
                               inputs["W2"], inputs["b2"])
    nc = _get_nc()
    in_maps = []
    for c in range(NCORES):
        shard = np.ascontiguousarray(
            x[c * BLOC:(c + 1) * BLOC].reshape(ROWS, D))
        in_maps.append(dict(x=shard, w1p=W1pk, c2v=c2, w2v=w2v))
    res = run_bass_kernel_spmd(nc, in_maps, core_ids=list(range(NCORES)),
                               trace=trace, **kw)
    full = np.concatenate([res.results[c]["out"] for c in range(NCORES)],
                          axis=0)
    return full, res


def kernel(**inputs) -> np.ndarray:
    out, _ = run_cores(inputs, trace=False)
    return out.astype(np.float32)
